# revision 1
# baseline (speedup 1.0000x reference)
"""MHA layer (QKV proj + masked softmax attention + out proj + residual + LayerNorm)
on 8 NeuronCores. Sharding: batch(4) x query-half(2). No collectives: each core
computes K/V for its full batch, Q only for its half of T.

Self-contained: hardcodes shapes from the problem spec.
"""

import numpy as np

import concourse.bass as bass
import concourse.bacc as bacc
import concourse.tile as tile
import concourse.mybir as mybir
from concourse.bass_utils import run_bass_kernel_spmd

B, T, C, H, D = 4, 2048, 1024, 16, 64
TQ = T // 2          # query rows per core
N_CORES = 8
P = 128
NJ = C // P          # 8 c-chunks
NTK = T // P         # 16 key chunks
LN_EPS = 1e-5
VSLOT = 66           # V_aug per-head slot: 64 V cols + 1 ones + 1 pad (4B align)

f32 = mybir.dt.float32
bf16 = mybir.dt.bfloat16
AX = mybir.AxisListType
ALU = mybir.AluOpType
ACTF = mybir.ActivationFunctionType


def build(affine: bool):
    import os as _os0
    phase_lim = int(_os0.environ.get("K_PHASE", "4"))
    nc = bacc.Bacc("TRN2", target_bir_lowering=False, debug=False,
                   num_devices=N_CORES)

    xbf = nc.dram_tensor("xbf", [T, C], bf16, kind="ExternalInput")
    w4 = nc.dram_tensor("w4", [4 * C, C], bf16, kind="ExternalInput")
    # fx rows: 0..TQ-1 xres; TQ+0 bq; +1 bk; +2 bv; +3 bp; +4 lng; +5 lnb; +6 mask
    fx = nc.dram_tensor("fx", [TQ + 7, C], f32, kind="ExternalInput")
    wq = w4[0 * C:1 * C, :]
    wk = w4[1 * C:2 * C, :]
    wv = w4[2 * C:3 * C, :]
    wp = w4[3 * C:4 * C, :]
    xres = fx[0:TQ, :]
    outd = nc.dram_tensor("out", [TQ, C], f32, kind="ExternalOutput")
    import os
    dbg = os.environ.get("K_DEBUG", "") == "1"
    if dbg:
        dbg_qt = nc.dram_tensor("dbg_qt", [P, TQ], bf16, kind="ExternalOutput")
        dbg_kt = nc.dram_tensor("dbg_kt", [P, T], bf16, kind="ExternalOutput")
        dbg_va = nc.dram_tensor("dbg_va", [P, H * VSLOT], bf16, kind="ExternalOutput")
        dbg_yt = nc.dram_tensor("dbg_yt", [P, TQ], bf16, kind="ExternalOutput")
        dbg_xt = nc.dram_tensor("dbg_xt", [P, T], bf16, kind="ExternalOutput")
        dbg_ex = nc.dram_tensor("dbg_ex", [P, TQ], bf16, kind="ExternalOutput")
        dbg_yu = nc.dram_tensor("dbg_yu", [65, TQ], f32, kind="ExternalOutput")
        dbg_sr = nc.dram_tensor("dbg_sr", [P, TQ], f32, kind="ExternalOutput")

    with tile.TileContext(nc) as tc:
        with (
            tc.tile_pool(name="pers", bufs=1) as pers,
            tc.tile_pool(name="big", bufs=1) as bigp,
            tc.tile_pool(name="wbig", bufs=1) as wbigp,
            tc.tile_pool(name="wsl", bufs=16) as wslp,
            tc.tile_pool(name="ev", bufs=2) as evp,
            tc.tile_pool(name="sm", bufs=2) as smp,
            tc.tile_pool(name="psum", bufs=1, space=bass.MemorySpace.PSUM) as psp,
        ):
            # ---- phase A: small loads, broadcasts, xT ----
            mrow_f = smp.tile([1, TQ], f32, tag="sr", name="mrow_f")
            nc.sync.dma_start(mrow_f[:], fx[TQ + 6:TQ + 7, :])
            mrow = pers.tile([1, TQ], bf16, tag="mrow")
            nc.vector.tensor_copy(mrow[:], mrow_f[:])
            bvrow = pers.tile([1, C], f32, tag="bvrow")
            nc.sync.dma_start(bvrow[:], fx[TQ + 2:TQ + 3, :])
            bprow = pers.tile([1, C], f32, tag="bprow")
            nc.sync.dma_start(bprow[:], fx[TQ + 3:TQ + 4, :])
            bq_t = pers.tile([P, NJ], f32, tag="bq_t")
            nc.sync.dma_start(bq_t[:],
                              fx[TQ + 0:TQ + 1, :].rearrange("a (j p) -> p (a j)", p=P))
            bk_t = pers.tile([P, NJ], f32, tag="bk_t")
            nc.sync.dma_start(bk_t[:],
                              fx[TQ + 1:TQ + 2, :].rearrange("a (j p) -> p (a j)", p=P))

            eps_t = pers.tile([P, 1], f32, tag="eps_t")
            nc.gpsimd.memset(eps_t[:], LN_EPS)
            mask_bc = pers.tile([P, TQ], bf16, tag="mask_bc")
            nc.gpsimd.partition_broadcast(mask_bc[:], mrow[:])
            bv_bc = pers.tile([P, C], f32, tag="bv_bc")
            nc.gpsimd.partition_broadcast(bv_bc[:], bvrow[:])
            bp_bc = pers.tile([P, C], f32, tag="bp_bc")
            nc.gpsimd.partition_broadcast(bp_bc[:], bprow[:])
            if affine:
                lngrow = pers.tile([1, C], f32, tag="lngrow")
                nc.sync.dma_start(lngrow[:], fx[TQ + 4:TQ + 5, :])
                lnbrow = pers.tile([1, C], f32, tag="lnbrow")
                nc.sync.dma_start(lnbrow[:], fx[TQ + 5:TQ + 6, :])
                lng_bc = pers.tile([P, C], f32, tag="lng_bc")
                nc.gpsimd.partition_broadcast(lng_bc[:], lngrow[:])
                lnb_bc = pers.tile([P, C], f32, tag="lnb_bc")
                nc.gpsimd.partition_broadcast(lnb_bc[:], lnbrow[:])

            # xT[j]: [128 (c-chunk j), T] bf16 via DMA xbar transpose from DRAM
            notr = _os0.environ.get("K_NOTR", "0") == "1"
            xt = []
            for j in range(NJ):
                t_ = bigp.tile([P, T], bf16, tag=f"xt{j}")
                if not notr:
                    nc.sync.dma_start_transpose(t_[:], xbf[:, j * P:(j + 1) * P])
                xt.append(t_)

            # ---- persistent attention operands ----
            qt = [pers.tile([P, TQ], bf16, tag=f"qt{j}", name=f"qt{j}")
                  for j in range(NJ)]
            kt = [pers.tile([P, T], bf16, tag=f"kt{j}", name=f"kt{j}")
                  for j in range(NJ)]
            vaug = [pers.tile([P, H * VSLOT], bf16, tag=f"va{t}", name=f"va{t}")
                    for t in range(NTK)]
            yt = [pers.tile([P, TQ], bf16, tag=f"yt{j}", name=f"yt{j}")
                  for j in range(NJ)]

            # ---- phase B1: V = x @ Wv + bv, into vaug (+ ones cols) ----
            if phase_lim >= 1:
                wv_sb = []
                for i in range(NJ):
                    w_ = wbigp.tile([P, C], bf16, tag=f"wbig{i}")
                    nc.sync.dma_start(w_[:], wv[i * P:(i + 1) * P, :])
                    wv_sb.append(w_)
                for tk in range(NTK):
                    ones_ap = vaug[tk][:].rearrange("p (h e) -> p h e", e=VSLOT)
                    nc.gpsimd.memset(ones_ap[:, :, 64:65], 1.0)
                for d2 in range(2):
                    for tk in range(NTK):
                        psv = psp.tile([P, 512], f32, tag="sc", bufs=4)
                        for i in range(NJ):
                            nc.tensor.matmul(
                                psv[:], xt[i][:, tk * P:(tk + 1) * P],
                                wv_sb[i][:, d2 * 512:(d2 + 1) * 512],
                                start=(i == 0), stop=(i == NJ - 1))
                        dst = vaug[tk][:].rearrange("p (h e) -> p h e", e=VSLOT)
                        nc.vector.tensor_tensor(
                            dst[:, 8 * d2:8 * d2 + 8, 0:64],
                            psv[:].rearrange("p (h d) -> p h d", d=D),
                            bv_bc[:, d2 * 512:(d2 + 1) * 512].rearrange(
                                "p (h d) -> p h d", d=D),
                            op=ALU.add)

            # ---- phase B2 + C: per c-chunk j: Q^T, K^T then attention ----
            def qk_produce(j):
                # Q^T chunk: [128 dq, TQ]
                wq_s = []
                wk_s = []
                for i in range(NJ):
                    w_ = wslp.tile([P, P], bf16, tag="wsl")
                    nc.sync.dma_start(w_[:], wq[i * P:(i + 1) * P,
                                                j * P:(j + 1) * P])
                    wq_s.append(w_)
                for i in range(NJ):
                    w_ = wslp.tile([P, P], bf16, tag="wsl")
                    nc.sync.dma_start(w_[:], wk[i * P:(i + 1) * P,
                                                j * P:(j + 1) * P])
                    wk_s.append(w_)
                for blk in range(2):
                    psq = psp.tile([P, 512], f32, tag="sc", bufs=4,
                                   name=f"psq{j}_{blk}")
                    for i in range(NJ):
                        nc.tensor.matmul(
                            psq[:], wq_s[i][:],
                            xt[i][:, blk * 512:(blk + 1) * 512],
                            start=(i == 0), stop=(i == NJ - 1))
                    # qt = (psq + bq) * mask (mask==0 rows -> 0 scores)
                    nc.vector.scalar_tensor_tensor(
                        qt[j][:, blk * 512:(blk + 1) * 512], psq[:],
                        bq_t[:, j:j + 1],
                        mask_bc[:, blk * 512:(blk + 1) * 512],
                        op0=ALU.add, op1=ALU.mult)
                for th in range(2):
                    for blk in range(2):
                        psk = psp.tile([P, 512], f32, tag="sc", bufs=4,
                                       name=f"psk{j}_{th}_{blk}")
                        for i in range(NJ):
                            nc.tensor.matmul(
                                psk[:], wk_s[i][:],
                                xt[i][:, th * 1024 + blk * 512:
                                         th * 1024 + (blk + 1) * 512],
                                start=(i == 0), stop=(i == NJ - 1))
                        nc.vector.tensor_scalar(
                            kt[j][:, th * 1024 + blk * 512:
                                     th * 1024 + (blk + 1) * 512], psk[:],
                            bk_t[:, j:j + 1], None, op0=ALU.add)


            def attn_chunk(j):
                # attention: both heads of chunk j, scores paired on PE
                yaccs = []
                for hh in range(2):
                    ya = psp.tile([65, TQ], f32, tag="yacc", bufs=2,
                                  name=f"yacc{j}_{hh}")
                    yaccs.append(ya)
                for tk in range(NTK):
                    exs = {}
                    for hh in range(2):
                        pb = hh * 64
                        for blk in range(2):
                            pss = psp.tile([P, 512], f32, tag="sc", bufs=4,
                                           name=f"pss{j}_{hh}_{blk}")
                            nc.tensor.matmul(
                                pss[:],
                                kt[j][pb:pb + 64, tk * P:(tk + 1) * P],
                                qt[j][pb:pb + 64, blk * 512:(blk + 1) * 512],
                                start=True, stop=True,
                                tile_position=(pb, 0))
                            ex = evp.tile([P, 512], bf16, tag="ex", bufs=6,
                                          name=f"ex{j}_{hh}_{blk}")
                            nc.scalar.activation(ex[:], pss[:], ACTF.Exp)
                            exs[(hh, blk)] = ex
                    for hh in range(2):
                        h = 2 * j + hh
                        for blk in range(2):
                            nc.tensor.matmul(
                                yaccs[hh][:, blk * 512:(blk + 1) * 512],
                                vaug[tk][:, h * VSLOT:h * VSLOT + 65],
                                exs[(hh, blk)][:],
                                start=(tk == 0), stop=(tk == NTK - 1))
                for hh in range(2):
                    yacc = yaccs[hh]
                    # normalize: row 64 of yacc is the softmax denominator
                    sr = smp.tile([P, TQ], f32, tag="sr")
                    if dbg and j == 0 and hh == 0:
                        nc.vector.tensor_copy(sr[0:65, :], yacc[:])
                        nc.sync.dma_start(dbg_yu[:], sr[0:65, :])
                    nc.vector.reciprocal(sr[64:65, :], yacc[64:65, :])
                    srb = smp.tile([1, TQ], f32, tag="srb", bufs=1)
                    nc.sync.dma_start(srb[:], sr[64:65, :])
                    nc.gpsimd.partition_broadcast(sr[0:64, :], srb[:])
                    if dbg and j == 0 and hh == 0:
                        nc.sync.dma_start(dbg_sr[:], sr[:])
                    if hh == 0:
                        nc.vector.tensor_tensor(
                            yt[j][0:64, :], yacc[0:64, :], sr[0:64, :],
                            op=ALU.mult)
                    else:
                        yo = smp.tile([64, TQ], bf16, tag="yo")
                        nc.vector.tensor_tensor(
                            yo[:], yacc[0:64, :], sr[0:64, :], op=ALU.mult)
                        nc.sync.dma_start(yt[j][64:128, :], yo[:])


            import os as _os
            prefix = _os.environ.get("K_PREFIX", "0") == "1"
            if phase_lim >= 2:
                if prefix or phase_lim < 3:
                    for j in range(NJ):
                        qk_produce(j)
                    if phase_lim >= 3:
                        for j in range(NJ):
                            attn_chunk(j)
                else:
                    for j in range(NJ):
                        qk_produce(j)
                        attn_chunk(j)
            # ---- phase D: out proj + residual + LayerNorm ----
            if phase_lim >= 4:
                wp_sb = []
                for i in range(NJ):
                    w_ = wbigp.tile([P, C], bf16, tag=f"wbig{i}")
                    nc.sync.dma_start(w_[:], wp[i * P:(i + 1) * P, :])
                    wp_sb.append(w_)
                for i in range(T // P // 2):  # 8 row-tiles of our TQ rows
                    xr = bigp.tile([P, C], f32, tag=f"xr{i % 2}", bufs=1,
                                   name=f"xr{i}")
                    nc.sync.dma_start(xr[:], xres[i * P:(i + 1) * P, :])
                    hres = evp.tile([P, C], f32, tag="hres", bufs=1)
                    for half in range(2):
                        pso = psp.tile([P, 512], f32, tag="sc", bufs=4,
                                       name=f"pso{i}_{half}")
                        for j in range(NJ):
                            nc.tensor.matmul(
                                pso[:],
                                yt[j][:, i * P:(i + 1) * P],
                                wp_sb[j][:, half * 512:(half + 1) * 512],
                                start=(j == 0), stop=(j == NJ - 1))
                        nc.vector.tensor_tensor(
                            hres[:, half * 512:(half + 1) * 512], pso[:],
                            bp_bc[:, half * 512:(half + 1) * 512], op=ALU.add)
                    nc.vector.tensor_tensor(hres[:], hres[:], xr[:], op=ALU.add)
                    stat = smp.tile([P, 8], f32, tag="stat")
                    nc.vector.reduce_sum(stat[:, 0:1], hres[:], axis=AX.X)
                    sq = evp.tile([P, C], f32, tag="sq", bufs=1)
                    nc.scalar.activation(sq[:], hres[:], ACTF.Square,
                                         accum_out=stat[:, 1:2])
                    # mu, m2, var
                    nc.vector.tensor_scalar(stat[:, 2:3], stat[:, 0:1],
                                            1.0 / C, None, op0=ALU.mult)
                    nc.vector.tensor_scalar(stat[:, 3:4], stat[:, 1:2],
                                            1.0 / C, None, op0=ALU.mult)
                    nc.vector.tensor_tensor(stat[:, 4:5], stat[:, 2:3],
                                            stat[:, 2:3], op=ALU.mult)
                    nc.vector.tensor_tensor(stat[:, 5:6], stat[:, 3:4],
                                            stat[:, 4:5], op=ALU.subtract)
                    nc.scalar.activation(stat[:, 6:7], stat[:, 5:6], ACTF.Sqrt,
                                         bias=eps_t[:])
                    nc.vector.reciprocal(stat[:, 7:8], stat[:, 6:7])
                    nc.vector.tensor_scalar(hres[:], hres[:], stat[:, 2:3],
                                            stat[:, 7:8], op0=ALU.subtract,
                                            op1=ALU.mult)
                    if affine:
                        nc.vector.tensor_tensor(hres[:], hres[:], lng_bc[:],
                                                op=ALU.mult)
                        nc.vector.tensor_tensor(hres[:], hres[:], lnb_bc[:],
                                                op=ALU.add)
                    nc.sync.dma_start(outd[i * P:(i + 1) * P, :], hres[:])

    nc.compile()
    return nc


_CACHE = {}


def _get_nc(affine: bool):
    if affine not in _CACHE:
        _CACHE[affine] = build(affine)
    return _CACHE[affine]


def _make_in_maps(x, Wq, bq, Wk, bk, Wv, bv, Wp, bp, ln_g, ln_b, mask,
                  affine: bool):
    bf = mybir.dt.np(bf16)
    sc = np.float32(1.0 / np.sqrt(D))
    w4_h = np.concatenate([
        np.asarray(Wq, np.float32) * sc, np.asarray(Wk, np.float32),
        np.asarray(Wv, np.float32), np.asarray(Wp, np.float32)],
        axis=0).astype(bf)
    x = np.asarray(x, np.float32)
    mask = np.asarray(mask)
    extra = np.stack([
        np.asarray(bq, np.float32) * sc, np.asarray(bk, np.float32),
        np.asarray(bv, np.float32), np.asarray(bp, np.float32),
        np.asarray(ln_g, np.float32), np.asarray(ln_b, np.float32),
        np.zeros(C, np.float32)], axis=0)
    in_maps = []
    for c in range(N_CORES):
        b, half = c // 2, c % 2
        xb = x[b]
        fx_h = np.empty((TQ + 7, C), np.float32)
        fx_h[0:TQ] = xb[half * TQ:(half + 1) * TQ]
        fx_h[TQ:] = extra
        fx_h[TQ + 6, :] = (mask[b, half * TQ:(half + 1) * TQ] != 0)
        m = {
            "xbf": np.roll(xb, -half * TQ, axis=0).astype(bf),
            "w4": w4_h,
            "fx": fx_h,
        }
        in_maps.append(m)
    return in_maps


def run(inputs: dict, trace: bool = False):
    ln_g = np.asarray(inputs["ln_g"], np.float32)
    ln_b = np.asarray(inputs["ln_b"], np.float32)
    affine = not (np.all(ln_g == 1.0) and np.all(ln_b == 0.0))
    nc = _get_nc(affine)
    in_maps = _make_in_maps(**inputs, affine=affine)
    res = None
    for attempt in range(3):
        try:
            res = run_bass_kernel_spmd(nc, in_maps, list(range(N_CORES)),
                                       trace=trace)
            break
        except Exception:
            if attempt == 2:
                raise
            import time as _time
            _time.sleep(2.0)
    out = np.empty((B, T, C), np.float32)
    for c in range(N_CORES):
        b, half = c // 2, c % 2
        out[b, half * TQ:(half + 1) * TQ] = res.results[c]["out"]
    return out, res


def kernel(**inputs) -> np.ndarray:
    out, _ = run(inputs, trace=False)
    return out

